# revision 14
# baseline (speedup 1.0000x reference)
"""Multi-head self-attention forward on 8 Trainium2 NeuronCores.

Problem: x[4, 2048, 1024] fp32, weights wq/wk/wv/wo [1024, 1024].
  Q,K,V = x @ w{q,k,v}.T (16 heads x 64); causal softmax(QK^T/8)V; out @ wo.T.

Sharding (single SPMD program, per-core data differs only):
  core c: batch b = c//2, head-half hh = c%2 (heads hh*8..hh*8+8),
  wo-half hh (output dims hh*512..). Per pair (2b, 2b+1):
    - each core: Q/K/V projections for its 8 heads (full 2048 tokens),
      causal flash attention for those heads, producing O^T [512, 2048]
    - pairwise AllGather of O^T -> O_full^T [1024, 2048]
    - each core: out-proj against its 512 output dims -> z [2048, 512]
  Host unshard: out[b][:, hh*512:] = core(2b+hh) output.

v3 design notes:
  - All transposes on the host: x^T, wq^T (pre-scaled by 1/8), wk^T,
    wv^T, wo^T and the 0/1 causal diag-block mask arrive as DRAM
    inputs. No PE transposes.
  - Scores are row-tiled: the two heads of a QK chunk occupy SBUF
    partitions 0:64 / 64:128 of the projection output, and their
    S^T = K_h @ Q_h^T matmuls run CONCURRENTLY on the two 64-row
    tiles of the PE array (tile_position (0,0) / (64,0)), reading the
    projection layout directly -- no zero-padding, no staging DMAs,
    and 2x score throughput vs a padded 128-contraction matmul.
  - Causal masking is a 0/1 multiply on the exp'd diag-block columns
    (DVE), replacing v1/v2's PE mask-seed + accumulate (which also
    trips CoreSim's coarse pending-zero model).
  - The softmax divide copies the replicated denominator out of PSUM
    and uses reciprocal_approx_fast (single custom-DVE pass, ~18
    bits) instead of the 8-cycle/element iterative RECIPROCAL (107us
    of DVE in v1).
  - Two-head lockstep + GK=2 psum_s groups double-buffered
    (2x2 banks) + psum_o (2) + projection psum (2) = 8 banks.
  - Q/K projection chunk c+1, the remaining V chains, and the g1
    out-proj chains are emitted between attention blocks (and via
    `mid` inside the diag group, between S and PV) so the in-order
    PE queue has independent work while ACT exps drain.
"""

import sys

sys.path.insert(0, "/opt/trn_rl_repo")

import ml_dtypes
import numpy as np
import concourse.bass as bass
import concourse.mybir as mybir
import concourse.tile as tile
from concourse import bacc
from concourse.bass_utils import run_bass_kernel_spmd

F32 = mybir.dt.float32
BF16 = mybir.dt.bfloat16
AF = mybir.ActivationFunctionType
OP = mybir.AluOpType

N_CORES = 8
S = 2048          # sequence length
D = 1024          # model dim
HL = 8            # heads per core
DK = 64           # head dim
DL = HL * DK      # local head dims = 512
GK = 2            # k-chunks per psum_s group

_NC_CACHE = {}


def build():
    nc = bacc.Bacc("TRN2", target_bir_lowering=False, debug=False, num_devices=N_CORES)

    xTd = nc.dram_tensor("xT", [D, S], BF16, kind="ExternalInput")
    wqTd = nc.dram_tensor("wqT", [D, DL], BF16, kind="ExternalInput")
    wkTd = nc.dram_tensor("wkT", [D, DL], BF16, kind="ExternalInput")
    wvTd = nc.dram_tensor("wvT", [D, DL], BF16, kind="ExternalInput")
    woTd = nc.dram_tensor("woT", [D, DL], BF16, kind="ExternalInput")
    trid = nc.dram_tensor("tri01", [128, 128], BF16, kind="ExternalInput")
    z = nc.dram_tensor("z", [S, DL], F32, kind="ExternalOutput")

    with tile.TileContext(nc) as tc:
        with (
            tc.tile_pool(name="cst", bufs=1) as cst,
            tc.tile_pool(name="per", bufs=1) as per,
            tc.tile_pool(name="dram", bufs=1, space="DRAM") as dram,
        ):
            # gpsimd-queue DMAs: keep the sync queue's critical prefix
            # (wv+x) minimal so the first V chains aren't gated on
            # unrelated bootstrap transfers
            tri01 = cst.tile([128, 128], BF16)
            nc.gpsimd.dma_start(tri01[:], trid[:])

            # persistent across attn -> out-proj
            OT = per.tile([128, 4, S], BF16)     # [p(dl in chunk), chunk, q]
            woTs = per.tile([128, 8, DL], BF16)  # [p(o in chunk), chunk, dout]
            for j in range(8):
                nc.gpsimd.dma_start(woTs[:, j, :], woTd[j * 128:(j + 1) * 128, :])

            cins = []
            gouts = []
            for j in range(4):
                cin_t = dram.tile([128, S], BF16, tag=f"cin{j}", name=f"cin{j}")
                gout_t = dram.tile([256, S], BF16, tag=f"gout{j}", name=f"gout{j}")
                cins.append(cin_t)
                gouts.append(gout_t)

            with (
                tc.tile_pool(name="attn", bufs=1) as attnp,
                tc.tile_pool(name="aps", bufs=2, space="PSUM") as aps,
                tc.tile_pool(name="apo", bufs=2, space="PSUM") as apo,
                tc.tile_pool(name="pps", bufs=2, space="PSUM") as pps,
            ):
                # Vaug: [p(tok in kc), h, kc, 0:64]=V, [.., 64:128]=ones
                # (the ones make PV emit the softmax denominator replicated
                # on psum partitions 64:128)
                VA = attnp.tile([128, 8, 16, 128], BF16)
                for h in range(8):
                    nc.gpsimd.memset(VA[:, h, :, 64:128], 1.0)

                def vchain(xT, wvTs, r):
                    pp = pps.tile([128, 512], F32, tag="pp")
                    for i in range(8):
                        nc.tensor.matmul(
                            pp[:], xT[:, i, r * 128:(r + 1) * 128], wvTs[:, i, :],
                            start=(i == 0), stop=(i == 7))
                    nc.vector.tensor_copy(
                        VA[:, 0:8, r, 0:64], pp.rearrange("p (h d) -> p h d", h=8))

                def qkchain(xT, wTs, c, dstT, tb):
                    pp = pps.tile([128, 512], F32, tag="pp")
                    for i in range(8):
                        nc.tensor.matmul(
                            pp[:], wTs[:, i, c * 128:(c + 1) * 128],
                            xT[:, i, tb * 512:(tb + 1) * 512],
                            start=(i == 0), stop=(i == 7))
                    nc.vector.tensor_copy(dstT[:, tb * 512:(tb + 1) * 512], pp[:])

                def attn_qb(c, qb, KTc, QTc, mid=()):
                    q0 = qb * 512
                    nkc = 4 * (qb + 1)
                    pos = {}
                    for h in (0, 1):
                        pos[h] = apo.tile([128, 512], F32, tag="po",
                                          name=f"po{h}")
                    for g0 in range(0, nkc, GK):
                        kcs = [g0, g0 + 1]
                        offs = [0, 512]
                        wss = [max(0, kc * 128 - q0) for kc in kcs]
                        diag = g0 >= 4 * qb
                        pss = {}
                        pts = {}
                        for h in (0, 1):
                            pss[h] = aps.tile([128, GK * 512], F32, tag="ps",
                                              name=f"ps{h}")
                            pts[h] = attnp.tile([128, GK * 512], BF16,
                                                tag="pt", bufs=4,
                                                name=f"pt{h}")
                        # S matmuls alternate row tiles T0/T8 so the two
                        # heads' 64-contraction matmuls run concurrently on
                        # the two halves of the PE array
                        for off, kc, ws in zip(offs, kcs, wss):
                            for h in (0, 1):
                                hp = slice(64 * h, 64 * h + 64)
                                nc.tensor.matmul(
                                    pss[h][:, off:off + 512 - ws],
                                    KTc[hp, kc * 128:(kc + 1) * 128],
                                    QTc[hp, q0 + ws:q0 + 512],
                                    start=True, stop=True,
                                    tile_position=(64 * h, 0))
                        for h in (0, 1):
                            ps, pt = pss[h], pts[h]
                            # exp per contiguous written run ([0,W0) and
                            # [512,512+W1) join only when W0 == 512)
                            if wss[0] == 0:
                                nc.scalar.activation(
                                    pt[:, 0:1024 - wss[1]],
                                    ps[:, 0:1024 - wss[1]], AF.Exp)
                            else:
                                nc.scalar.activation(
                                    pt[:, 0:512 - wss[0]],
                                    ps[:, 0:512 - wss[0]], AF.Exp)
                                nc.scalar.activation(
                                    pt[:, 512:1024 - wss[1]],
                                    ps[:, 512:1024 - wss[1]], AF.Exp)
                            if diag:
                                for off in offs:
                                    nc.vector.tensor_tensor(
                                        pt[:, off:off + 128],
                                        pt[:, off:off + 128], tri01[:], OP.mult)
                        if diag:
                            for fn in mid:
                                fn()
                            mid = ()
                        for h in (0, 1):
                            hl = 2 * c + h
                            for off, kc in zip(offs, kcs):
                                ws = max(0, kc * 128 - q0)
                                nc.tensor.matmul(
                                    pos[h][:, ws:512],
                                    VA[:, hl, kc, :],
                                    pts[h][:, off:off + 512 - ws],
                                    start=(kc == 0), stop=(kc == nkc - 1))
                    for h in (0, 1):
                        hl = 2 * c + h
                        dnm = attnp.tile([64, 512], F32, tag="dnm", bufs=2)
                        rec = attnp.tile([64, 512], F32, tag="rec", bufs=2)
                        nc.vector.tensor_copy(dnm[:], pos[h][64:128, :])
                        nc.vector.reciprocal_approx_fast(rec[:], dnm[:])
                        nc.vector.tensor_tensor(
                            OT[(hl % 2) * 64:(hl % 2) * 64 + 64, hl // 2,
                               q0:q0 + 512],
                            pos[h][0:64, :], rec[:], OP.mult)

                def gather(c):
                    nc.sync.dma_start(cins[c][:], OT[:, c, :])
                    nc.gpsimd.collective_compute(
                        "AllGather", OP.bypass,
                        replica_groups=[[0, 1], [2, 3], [4, 5], [6, 7]],
                        ins=[cins[c][:]], outs=[gouts[c][:]])

                def alloc_kq():
                    KTc = attnp.tile([128, S], BF16, tag="KT", bufs=2)
                    QTc = attnp.tile([128, S], BF16, tag="QT", bufs=2)
                    return KTc, QTc

                with tc.tile_pool(name="wsb", bufs=1) as wsb:
                    xT = wsb.tile([128, 8, S], BF16)   # [p(din in chunk), chunk, tok]
                    wvTs = wsb.tile([128, 8, DL], BF16)
                    wkTs = wsb.tile([128, 8, DL], BF16)
                    wqTs = wsb.tile([128, 8, DL], BF16)
                    # DMA emission order = dependency order: the first
                    # chains' coarse DMA semaphore only covers transfers
                    # emitted before them
                    for i in range(8):
                        nc.sync.dma_start(wvTs[:, i, :], wvTd[i * 128:(i + 1) * 128, :])
                    for i in range(8):
                        nc.sync.dma_start(xT[:, i, :], xTd[i * 128:(i + 1) * 128, :])
                    for i in range(8):
                        nc.sync.dma_start(wkTs[:, i, :], wkTd[i * 128:(i + 1) * 128, :])

                    # ---- fill: just K0/Q0 over the first query block;
                    # V r0..3 lands inside block (0,0) between S and PV ----
                    KT0, QT0 = alloc_kq()
                    qkchain(xT, wkTs, 0, KT0, 0)
                    for i in range(8):
                        nc.sync.dma_start(wqTs[:, i, :], wqTd[i * 128:(i + 1) * 128, :])
                    qkchain(xT, wqTs, 0, QT0, 0)

                    # ---- pair 0: V chains + remaining K0/Q0 + chunk 1 ----
                    KT1, QT1 = alloc_kq()
                    attn_qb(0, 0, KT0, QT0,
                            mid=[lambda: vchain(xT, wvTs, 0),
                                 lambda: vchain(xT, wvTs, 1),
                                 lambda: vchain(xT, wvTs, 2),
                                 lambda: vchain(xT, wvTs, 3)])
                    qkchain(xT, wkTs, 0, KT0, 1)
                    qkchain(xT, wqTs, 0, QT0, 1)
                    vchain(xT, wvTs, 4)
                    vchain(xT, wvTs, 5)
                    attn_qb(0, 1, KT0, QT0,
                            mid=[lambda: vchain(xT, wvTs, 6),
                                 lambda: vchain(xT, wvTs, 7)])
                    qkchain(xT, wkTs, 0, KT0, 2)
                    qkchain(xT, wqTs, 0, QT0, 2)
                    vchain(xT, wvTs, 8)
                    vchain(xT, wvTs, 9)
                    attn_qb(0, 2, KT0, QT0,
                            mid=[lambda: vchain(xT, wvTs, 10),
                                 lambda: vchain(xT, wvTs, 11)])
                    qkchain(xT, wkTs, 0, KT0, 3)
                    qkchain(xT, wqTs, 0, QT0, 3)
                    vchain(xT, wvTs, 12)
                    vchain(xT, wvTs, 13)
                    attn_qb(0, 3, KT0, QT0,
                            mid=[lambda: vchain(xT, wvTs, 14),
                                 lambda: vchain(xT, wvTs, 15)])
                    for tb in range(4):
                        qkchain(xT, wkTs, 1, KT1, tb)
                    for tb in range(4):
                        qkchain(xT, wqTs, 1, QT1, tb)
                    gather(0)

                    # ---- pairs 1, 2: next chunk interleaved ----
                    KQ = {1: (KT1, QT1)}
                    for c in (1, 2):
                        nx = c + 1
                        KTc, QTc = KQ[c]
                        KTn, QTn = alloc_kq()
                        KQ[nx] = (KTn, QTn)
                        attn_qb(c, 0, KTc, QTc,
                                mid=[lambda: qkchain(xT, wkTs, nx, KTn, 0)])
                        qkchain(xT, wkTs, nx, KTn, 1)
                        attn_qb(c, 1, KTc, QTc,
                                mid=[lambda: qkchain(xT, wkTs, nx, KTn, 2)])
                        qkchain(xT, wkTs, nx, KTn, 3)
                        attn_qb(c, 2, KTc, QTc,
                                mid=[lambda: qkchain(xT, wqTs, nx, QTn, 0)])
                        qkchain(xT, wqTs, nx, QTn, 1)
                        attn_qb(c, 3, KTc, QTc,
                                mid=[lambda: qkchain(xT, wqTs, nx, QTn, 2)])
                        qkchain(xT, wqTs, nx, QTn, 3)
                        gather(c)

                # ---- pair 3 + out-proj (reuses the freed wsb space) ----
                with (
                    tc.tile_pool(name="otf", bufs=1) as otfp,
                    tc.tile_pool(name="zt1p", bufs=1) as zt1p,
                    tc.tile_pool(name="zsb", bufs=3) as zsb,
                ):
                    otf = {}

                    def otf_dma(j):
                        # scalar-queue DMA: keeps the sync queue free for the
                        # cin/z stream and can't block pair-3's exps (emitted
                        # after them)
                        src, row = (gouts[j], 0) if j < 4 else (gouts[j - 4], 128)
                        ofr = otfp.tile([128, S], BF16, tag=f"otf{j}",
                                        name=f"otf{j}")
                        nc.scalar.dma_start(ofr[:], src[row:row + 128, :])
                        otf[j] = ofr

                    g1js = [0, 1, 2, 4, 5, 6]
                    g2js = [3, 7]
                    zt1s = {}

                    def g1chain(qt):
                        pz = pps.tile([128, DL], F32, tag="pp")
                        for n, j in enumerate(g1js):
                            nc.tensor.matmul(
                                pz[:], otf[j][:, qt * 128:(qt + 1) * 128],
                                woTs[:, j, :],
                                start=(n == 0), stop=(n == len(g1js) - 1))
                        zt1 = zt1p.tile([128, DL], BF16, tag=f"zt1_{qt}",
                                        name=f"zt1_{qt}")
                        nc.vector.tensor_copy(zt1[:], pz[:])
                        zt1s[qt] = zt1

                    # pair-3 attention stays decoupled from the gathers: the
                    # g1 chains wait on otf DMAs (<- AllGathers), and any
                    # inter-core skew there would stall the in-order PE
                    # queue, so they run after the last attention block while
                    # gather(3)'s rendezvous proceeds on the CC stream.
                    KT3, QT3 = KQ[3]
                    attn_qb(3, 0, KT3, QT3)
                    attn_qb(3, 1, KT3, QT3)
                    attn_qb(3, 2, KT3, QT3)
                    attn_qb(3, 3, KT3, QT3)
                    gather(3)
                    for j in g1js:
                        otf_dma(j)
                    for qt in range(16):
                        g1chain(qt)
                    for j in g2js:
                        otf_dma(j)

                    for qt in range(16):
                        pz = pps.tile([128, DL], F32, tag="pp")
                        for n, j in enumerate(g2js):
                            nc.tensor.matmul(
                                pz[:], otf[j][:, qt * 128:(qt + 1) * 128],
                                woTs[:, j, :],
                                start=(n == 0), stop=(n == len(g2js) - 1))
                        zt = zsb.tile([128, DL], F32, tag="zt")
                        nc.vector.tensor_tensor(zt[:], pz[:], zt1s[qt][:], OP.add)
                        nc.sync.dma_start(z[qt * 128:(qt + 1) * 128, :], zt[:])

    nc.compile()
    return nc


def _get_nc():
    if "nc" not in _NC_CACHE:
        _NC_CACHE["nc"] = build()
    return _NC_CACHE["nc"]


def make_inputs(x, wq, wk, wv, wo):
    bf = ml_dtypes.bfloat16
    tri01 = np.where(np.arange(128)[:, None] <= np.arange(128)[None, :],
                     np.float32(1.0), np.float32(0.0)).astype(bf)
    xTs = [np.ascontiguousarray(x[bi].T).astype(bf) for bi in range(4)]
    whalf = {}
    for hh in range(2):
        sl = slice(hh * DL, (hh + 1) * DL)
        whalf[hh] = {
            "wqT": np.ascontiguousarray((wq[sl, :] * 0.125).T).astype(bf),
            "wkT": np.ascontiguousarray(wk[sl, :].T).astype(bf),
            "wvT": np.ascontiguousarray(wv[sl, :].T).astype(bf),
            "woT": np.ascontiguousarray(wo[sl, :].T).astype(bf),
        }
    in_maps = []
    for c in range(N_CORES):
        bi, hh = c // 2, c % 2
        m = {"xT": xTs[bi], "tri01": tri01}
        m.update(whalf[hh])
        in_maps.append(m)
    return in_maps


def kernel(x, wq, wk, wv, wo, _trace=False):
    x = np.asarray(x, dtype=np.float32)
    wq = np.asarray(wq, dtype=np.float32)
    wk = np.asarray(wk, dtype=np.float32)
    wv = np.asarray(wv, dtype=np.float32)
    wo = np.asarray(wo, dtype=np.float32)
    b, s, d = x.shape
    assert (b, s, d) == (4, S, D)

    in_maps = make_inputs(x, wq, wk, wv, wo)
    nc = _get_nc()
    res = run_bass_kernel_spmd(nc, in_maps, core_ids=list(range(N_CORES)),
                               trace=_trace)

    out = np.empty((4, S, D), dtype=np.float32)
    for c in range(N_CORES):
        bi, hh = c // 2, c % 2
        out[bi][:, hh * DL:(hh + 1) * DL] = res.results[c]["z"]
    if _trace:
        kernel.last_exec_time_ns = res.exec_time_ns
    return out
